# revision 15
# baseline (speedup 1.0000x reference)
"""Trainium2 Bass kernel for nn_MAR_52209622450490 (OctFormer sparse attention).

Sharding: depth2batch applied host-side while sharding — each core gets a
contiguous 2048-token slice of the window-ordered token stream (multiple of
the 512-token super-window), so all window partitions are core-local; cores
emit 4 partial sums combined on host.

v2 layout (vs the 404 us baseline):
 - fp8e4 DoubleRow matmuls for QKV/V/fc1/fc2/attn-proj/AV: one DR matmul
   does the 2-chunk contraction at 0.5 cyc/row (4x less PE time). Post-LN
   activations are written fp8 channel-PAIR-packed and DMA-transposed as
   uint16 (XBAR needs 2-byte), so HT partitions hold channel pairs (2j,2j+1)
   matching host-side W.reshape(128,2,*) DoubleRow stationary layout.
 - LN rstd via DVE tensor_scalar pow(-0.5) — no ACT Sqrt, no table churn.
   ACT only runs Exp / Gelu / Ln -> ~2 table loads per block.
 - scores exp over 2-bank [128,1024] PSUM tiles (halved per-inst overhead);
   a fraction of exps run on DVE as pow(e, x) broadcast, balancing ACT/DVE.
 - Pool (GPSIMD) takes residual adds, V evac, O-normalize, embed add.
 - scores stay bf16 row-tiled (K=32 strips); softmax normalizer via ones
   column in V, normalization fused into the O evac on Pool.
"""
import numpy as np
import ml_dtypes

import concourse.tile as tile
from concourse import bacc, mybir
from concourse.bass_utils import run_bass_kernel_spmd
from concourse.masks import make_identity

N_SPLIT = 4096
N_VQ = 12288
N = N_SPLIT + N_VQ
C = 256
H = 8
DH = 32
L = 4
P = 256
DIL = 2
HID = 4 * C
VQ_G = 4
VQ_SIZE = 256
NCORES = 8
T = N // NCORES            # 2048 tokens per core
TC = T // 128              # 16 row-tiles per core
NWIN = T // P              # 8 windows per core
EPS = 1e-5
SCALE = DH ** -0.5

F32 = mybir.dt.float32
BF16 = mybir.dt.bfloat16
U16 = mybir.dt.uint16
FP8 = mybir.dt.float8e4
BF = ml_dtypes.bfloat16
F8 = ml_dtypes.float8_e4m3
DR = mybir.MatmulPerfMode.DoubleRow

_CACHE = {}

# engine knobs
EXP_DVE_MOD = 4        # every Nth exp tile runs on DVE (pow) instead of ACT


def _sin_pos_emb(n, c):
    pos = np.arange(n, dtype=np.float32)[:, None]
    half = c // 2
    freqs = np.exp(-np.log(10000.0) * np.arange(half, dtype=np.float32) / half)
    ang = pos * freqs
    return np.concatenate([np.sin(ang), np.cos(ang)], axis=-1).astype(np.float32)


def _st(beg, cnt, step):
    return slice(beg, beg + (cnt - 1) * step + 1, step)


from contextlib import ExitStack as _ES

PRIO_OFF = 700


def build_nc(flags, n_blocks=L, dump=None):
    """flags: dict name->bool, whether each bias family is nonzero."""
    nc = bacc.Bacc(None, target_bir_lowering=False)

    d_emb = nc.declare_dram_parameter("emb", [T, C], F32, isOutput=False)
    d_zqt = nc.declare_dram_parameter("zqt", [DH, T], BF16, isOutput=False)
    d_vqpw = nc.declare_dram_parameter("vqpw", [DH, C], BF16, isOutput=False)
    # DoubleRow weights: [128, 2, M] with row (j, b) = channel 2j+b
    d_wqkv = nc.declare_dram_parameter("wqkv", [L, 128, 2, 3 * C], FP8, isOutput=False)
    d_wattn = nc.declare_dram_parameter("wattn", [L, 128, 2, C], FP8, isOutput=False)
    d_wfc1 = nc.declare_dram_parameter("wfc1", [L, 128, 2, HID], FP8, isOutput=False)
    d_wfc2 = nc.declare_dram_parameter("wfc2", [L, HID, C], FP8, isOutput=False)
    d_bqkv = nc.declare_dram_parameter("bqkv", [L, 3 * C], F32, isOutput=False)
    d_battn = nc.declare_dram_parameter("battn", [L, C], F32, isOutput=False)
    d_bfc1 = nc.declare_dram_parameter("bfc1", [L, HID], F32, isOutput=False)
    d_bfc2 = nc.declare_dram_parameter("bfc2", [L, C], F32, isOutput=False)
    d_wvq = nc.declare_dram_parameter("wvq", [C, VQ_G * VQ_SIZE], BF16, isOutput=False)
    d_wspl = nc.declare_dram_parameter("wspl", [C, 2], BF16, isOutput=False)
    d_bspl = nc.declare_dram_parameter("bspl", [2], F32, isOutput=False)
    d_ebq = nc.declare_dram_parameter("ebq", [VQ_G * VQ_SIZE], F32, isOutput=False)
    d_wsel = nc.declare_dram_parameter("wsel", [T, C], BF16, isOutput=False)
    d_bsel = nc.declare_dram_parameter("bsel", [T], F32, isOutput=False)
    d_msc = nc.declare_dram_parameter("msc", [T], F32, isOutput=False)
    d_mvc = nc.declare_dram_parameter("mvc", [T], F32, isOutput=False)
    d_stc = nc.declare_dram_parameter("stc", [T], F32, isOutput=False)
    d_out = nc.declare_dram_parameter("out", [128, 4], F32, isOutput=True)
    d_dbg = None
    if dump is not None:
        d_dbg = nc.declare_dram_parameter("dbg", [T, C], F32, isOutput=True)

    exp_ctr = [0]

    with tile.TileContext(nc) as tc:
        with (
            tc.tile_pool(name="big", bufs=1) as big,
            tc.tile_pool(name="wpool", bufs=2) as wp,
            tc.tile_pool(name="small", bufs=1) as sm,
            tc.tile_pool(name="trans", bufs=10) as tr,
            tc.tile_pool(name="ebpool", bufs=5) as ebp,
            tc.tile_pool(name="evpool", bufs=3) as evp,
            tc.tile_pool(name="psum", bufs=2, space="PSUM") as psp,
            tc.tile_pool(name="psum_sc", bufs=2, space="PSUM") as psc,
            tc.tile_pool(name="psum_av", bufs=2, space="PSUM") as pav,
        ):
            XB = big.tile([128, TC, C], F32, tag="XB")
            HTp = big.tile([128, T], U16, tag="HTp")      # fp8 channel-pair packed
            QT = big.tile([128, 2, T], BF16, tag="QT")
            KT = big.tile([128, 2, T], BF16, tag="KT")
            VB = big.tile([128, TC, H, DH + 2], FP8, tag="VB")
            OTp = big.tile([128, T], U16, tag="OTp")      # proj input packed
            GT = big.tile([128, HID // 128, T], FP8, tag="GT")
            XN = big.tile([128, TC, C], BF16, tag="XN")

            MHALF = sm.tile([128, 1], F32, tag="mhalf")
            nc.vector.memset(MHALF[:], -0.5)
            zqt = sm.tile([DH, T], BF16, tag="zqt")
            nc.sync.dma_start(zqt[:], d_zqt[:])
            vqpw = sm.tile([DH, C], BF16, tag="vqpw")
            nc.sync.dma_start(vqpw[:], d_vqpw[:])

            nc.vector.memset(VB[:, :, :, DH], 1.0)

            def HTv():
                """HTp as DoubleRow rhs/lhsT view: [128, 2, T] fp8."""
                return HTp[:].bitcast(FP8).rearrange("p (t b) -> p b t", b=2)

            def OTv():
                return OTp[:].bitcast(FP8).rearrange("p (t b) -> p b t", b=2)

            # ---------------- embed ----------------
            _embv = d_emb.rearrange("(t p) c -> p t c", p=128)
            for tq in range(4):
                nc.sync.dma_start(XB[:, tq * 4:(tq + 1) * 4, :],
                                  _embv[:, tq * 4:(tq + 1) * 4, :])
            for t in range(TC):
                ps = psp.tile([128, 512], F32, tag="bank")
                nc.tensor.matmul(ps[:, :C], zqt[:, t * 128:(t + 1) * 128],
                                 vqpw[:], start=True, stop=True)
                nc.vector.tensor_tensor(XB[:, t, :], XB[:, t, :], ps[:, :C],
                                        mybir.AluOpType.add)

            def layernorm_to(dst, t):
                """LN apply of XB chunk t into dst (any dtype, [128, C])."""
                st6 = tr.tile([128, 6], F32, tag="bn6")
                nc.vector.bn_stats(st6[:], XB[:, t, :])
                mv2 = tr.tile([128, 2], F32, tag="bn2")
                nc.vector.bn_aggr(mv2[:], st6[:])
                vpe = tr.tile([128, 1], F32, tag="vpe")
                nc.gpsimd.tensor_scalar_add(vpe[:], mv2[:, 1:2], EPS)
                rstd = tr.tile([128, 1], F32, tag="rstd")
                nc.gpsimd.tensor_tensor(rstd[:], vpe[:], MHALF[:],
                                        mybir.AluOpType.pow)
                nc.gpsimd.tensor_scalar(dst, XB[:, t, :],
                                        mv2[:, 0:1], rstd[:],
                                        mybir.AluOpType.subtract,
                                        mybir.AluOpType.mult)

            def ln_to_htp(t):
                h8 = tr.tile([128, C], FP8, tag="h8")
                layernorm_to(h8[:], t)
                nc.sync.dma_start_transpose(
                    HTp[:, t * 128:(t + 1) * 128], h8[:].bitcast(U16))

            def exp_tile(out_ap, in_ap):
                """exp over one flat [128, n] view; alternates ACT / DVE pow."""
                exp_ctr[0] += 1
                nc.scalar.activation(out_ap, in_ap,
                                     mybir.ActivationFunctionType.Exp)

            def prio(cond):
                s = _ES()
                if cond:
                    s.enter_context(tc.high_priority(offset=PRIO_OFF))
                return s

            # ---------------- transformer blocks ----------------
            for l in range(n_blocks):
              with prio(l > 0):
                wqkv = wp.tile([128, 2, 3 * C], FP8, tag="wqkv")
                nc.sync.dma_start(wqkv[:], d_wqkv[l])
                wattn = wp.tile([128, 2, C], FP8, tag="wattn")
                nc.sync.dma_start(wattn[:], d_wattn[l])
                wfc1 = wp.tile([128, 2, HID], FP8, tag="wfc1")
                nc.sync.dma_start(wfc1[:], d_wfc1[l])
                wfc2 = wp.tile([128, HID // 128, C], FP8, tag="wfc2")
                nc.sync.dma_start(wfc2[:], d_wfc2[l].rearrange("(k p) n -> p k n", p=128))
                bqkv = vbb = abb = f1b = f2b = None
                if flags["bqkv"]:
                    bqkv = wp.tile([128, 4], F32, tag="bqkv")
                    nc.sync.dma_start(bqkv[:], d_bqkv[l, :2 * C].rearrange("(g p) -> p g", p=128))
                if flags["bqkv_v"]:
                    vbb = wp.tile([128, C], F32, tag="vbb")
                    nc.sync.dma_start(vbb[:], d_bqkv[l, 2 * C:].to_broadcast([128, C]))
                if flags["battn"]:
                    abb = wp.tile([128, C], F32, tag="abb")
                    nc.sync.dma_start(abb[:], d_battn[l].to_broadcast([128, C]))
                if flags["bfc1"]:
                    f1b = wp.tile([128, HID // 128], F32, tag="f1b")
                    nc.sync.dma_start(f1b[:], d_bfc1[l].rearrange("(g p) -> p g", p=128))
                if flags["bfc2"]:
                    f2b = wp.tile([128, C], F32, tag="f2b")
                    nc.sync.dma_start(f2b[:], d_bfc2[l].to_broadcast([128, C]))

                dil = DIL if (l % 2 == 1) else 1

                # LN1 -> fp8-packed HTp (first chunks priority-backdated)
                with prio(True):
                    for t in range(4):
                        ln_to_htp(t)
                for t in range(4, TC):
                    ln_to_htp(t)

                # QT / KT via DoubleRow (transposed: out partitions = qk dims);
                # Q-pair / K-pair share a 2-bank PSUM tile, one evac each.
                for nk in range(T // 512):
                    with prio(nk == 0):
                        for qk in range(2):     # 0 -> Q ; 1 -> K
                            dstT = QT if qk == 0 else KT
                            pss = psc.tile([128, 1024], F32, tag="ps_sc")
                            for gg in range(2):
                                g = qk * 2 + gg
                                nc.tensor.matmul(
                                    pss[:, gg * 512:(gg + 1) * 512],
                                    wqkv[:, :, g * 128:(g + 1) * 128],
                                    HTv()[:, :, nk * 512:(nk + 1) * 512],
                                    start=True, stop=True, perf_mode=DR)
                            dsl = dstT[:, :, nk * 512:(nk + 1) * 512]
                            src = pss[:].rearrange("p (g x) -> p g x", g=2)
                            if flags["bqkv"]:
                                for gg in range(2):
                                    nc.vector.tensor_scalar(
                                        dsl[:, gg, :], src[:, gg, :],
                                        bqkv[:, qk * 2 + gg:qk * 2 + gg + 1],
                                        None, mybir.AluOpType.add)
                            elif nk == 0 or (nk + qk) % 2 == 0:
                                nc.scalar.activation(
                                    dsl, src,
                                    mybir.ActivationFunctionType.Identity)
                            else:
                                nc.vector.tensor_copy(dsl, src)

                # V in window order: VB[:, wlin*2+qc, h, 0:DH]
                for wlin in range(NWIN):
                  with prio(wlin < 2):
                    sw, r = divmod(wlin, dil)
                    start = sw * P * dil + r
                    for qc in range(2):
                        ps = psp.tile([128, 512], F32, tag="bank")
                        tok = _st(start + qc * 128 * dil, 128, dil)
                        for bb in range(2):
                            nc.tensor.matmul(ps[:, :C], HTv()[:, bb, tok],
                                             wqkv[:, bb, 2 * C:3 * C],
                                             start=(bb == 0), stop=(bb == 1))
                        vdst = VB[:, wlin * 2 + qc, :, 0:DH]
                        psv = ps[:, :C].rearrange("p (h d) -> p h d", h=H)
                        if flags["bqkv_v"]:
                            nc.vector.tensor_tensor(
                                vdst, psv,
                                vbb[:].rearrange("p (h d) -> p h d", h=H),
                                mybir.AluOpType.add)
                        elif wlin < 2:
                            nc.scalar.activation(
                                vdst, psv,
                                mybir.ActivationFunctionType.Identity)
                        else:
                            nc.vector.tensor_copy(vdst, psv)

                # attention. scores bf16 row-tiled; exp over 2-bank tiles;
                # EB layout [128, hp, kc, (hs2|q)] fp8 for DoubleRow AV.
                def scores_for(wlin):
                    sw, r = divmod(wlin, dil)
                    start = sw * P * dil + r
                    alltok = _st(start, P, dil)
                    EB = ebp.tile([128, 4, 2, 512], FP8, tag="EB")
                    for g in range(2):
                        for hp in range(2):
                            pss = psc.tile([128, 1024], F32, tag="ps_sc")
                            for hs2 in range(2):
                                hs = hp * 2 + hs2
                                prt = slice(hs * 32, (hs + 1) * 32)
                                for kc in range(2):
                                    ktok = _st(start + kc * 128 * dil, 128, dil)
                                    nc.tensor.matmul(
                                        pss[:, kc * 512 + hs2 * 256:
                                            kc * 512 + hs2 * 256 + 256],
                                        KT[prt, g, ktok],
                                        QT[prt, g, alltok],
                                        start=True, stop=True,
                                        tile_position=(hs * 32, 0))
                            exp_tile(EB[:, g * 2 + hp, :, :].rearrange(
                                         "p a b -> p (a b)"),
                                     pss[:])
                    return EB

                EBs = {}
                for wlin in range(NWIN):
                    if wlin % 2 == 0:
                        with prio(wlin == 0):
                            EBs[wlin] = scores_for(wlin)
                            EBs[wlin + 1] = scores_for(wlin + 1)
                    sw, r = divmod(wlin, dil)
                    start = sw * P * dil + r
                    EB = EBs[wlin]
                    for wc in range(2):
                        pso = pav.tile([128, H, DH + 1], F32, tag="ps_av")
                        for hh in range(H):
                            hp, hs2 = divmod(hh, 2)
                            nc.tensor.matmul(
                                pso[:, hh, :],
                                EB[:, hp, :, hs2 * 256 + wc * 128:
                                   hs2 * 256 + wc * 128 + 128],
                                VB[:, wlin * 2:wlin * 2 + 2, hh, 0:DH + 1],
                                start=True, stop=True, perf_mode=DR)
                        rz = tr.tile([128, H], F32, tag="rz")
                        nc.vector.reciprocal(rz[:], pso[:, :, DH])
                        o8 = tr.tile([128, C], FP8, tag="o8")
                        nc.vector.tensor_tensor(
                            o8[:].rearrange("p (h d) -> p h d", h=H),
                            pso[:, :, 0:DH],
                            rz[:, :, None].to_broadcast([128, H, DH]),
                            mybir.AluOpType.mult)
                        if dil == 1:
                            nc.sync.dma_start_transpose(
                                OTp[:, start + wc * 128:start + wc * 128 + 128],
                                o8[:].bitcast(U16))
                        else:
                            stg = tr.tile([128, 128], U16, tag="stg")
                            nc.sync.dma_start_transpose(stg[:], o8[:].bitcast(U16))
                            dtok = _st(start + wc * 128 * dil, 128, dil)
                            nc.sync.dma_start(OTp[:, dtok], stg[:])

                # proj + residual + LN2 + repack, fused per t-chunk
                for t in range(TC):
                    ps = psp.tile([128, 512], F32, tag="bank")
                    for bb in range(2):
                        nc.tensor.matmul(ps[:, :C],
                                         OTv()[:, bb, t * 128:(t + 1) * 128],
                                         wattn[:, bb, :],
                                         start=(bb == 0), stop=(bb == 1))
                    nc.vector.tensor_tensor(XB[:, t, :], XB[:, t, :], ps[:, :C],
                                            mybir.AluOpType.add)
                    if flags["battn"]:
                        nc.gpsimd.tensor_tensor(XB[:, t, :], XB[:, t, :], abb[:],
                                                mybir.AluOpType.add)
                    with prio(t < 4):
                        ln_to_htp(t)

                # fc1 + gelu -> GT (2-bank batches), then fc2 + residual
                for nk in range(T // 512):
                  with prio(nk < 2):
                    for mp in range(HID // 256):
                        pss = psc.tile([128, 1024], F32, tag="ps_sc")
                        for m2 in range(2):
                            nc.tensor.matmul(
                                pss[:, m2 * 512:(m2 + 1) * 512],
                                wfc1[:, :, (mp * 2 + m2) * 128:(mp * 2 + m2 + 1) * 128],
                                HTv()[:, :, nk * 512:(nk + 1) * 512],
                                start=True, stop=True, perf_mode=DR)
                        gdst = GT[:, mp * 2:mp * 2 + 2, nk * 512:(nk + 1) * 512]
                        gsrc = pss[:].rearrange("p (m x) -> p m x", m=2)
                        if flags["bfc1"]:
                            for m2 in range(2):
                                nc.scalar.activation(
                                    gdst[:, m2, :], gsrc[:, m2, :],
                                    mybir.ActivationFunctionType.Gelu_apprx_tanh,
                                    bias=f1b[:, mp * 2 + m2:mp * 2 + m2 + 1])
                        else:
                            nc.scalar.activation(
                                gdst, gsrc,
                                mybir.ActivationFunctionType.Gelu_apprx_tanh)
                for t in range(TC):
                    ps = psp.tile([128, 512], F32, tag="bank")
                    for hp in range(HID // 256):
                        nc.tensor.matmul(ps[:, :C],
                                         GT[:, 2 * hp:2 * hp + 2, t * 128:(t + 1) * 128],
                                         wfc2[:, 2 * hp:2 * hp + 2, :],
                                         start=(hp == 0), stop=(hp == HID // 256 - 1),
                                         perf_mode=DR)
                    nc.vector.tensor_tensor(XB[:, t, :], XB[:, t, :], ps[:, :C],
                                            mybir.AluOpType.add)
                    if flags["bfc2"]:
                        nc.gpsimd.tensor_tensor(XB[:, t, :], XB[:, t, :], f2b[:],
                                                mybir.AluOpType.add)

            if dump == "xb":
                nc.sync.dma_start(d_dbg.rearrange("(t p) c -> p t c", p=128), XB[:])

            # ---------------- heads ----------------
            wvq = sm.tile([128, 2, VQ_G * VQ_SIZE], BF16, tag="wvq")
            nc.sync.dma_start(wvq[:], d_wvq.rearrange("(k p) n -> p k n", p=128))
            wspl = sm.tile([128, 2, 2], BF16, tag="wspl")
            nc.sync.dma_start(wspl[:], d_wspl.rearrange("(k p) n -> p k n", p=128))
            wselB = sm.tile([128, TC, C], BF16, tag="wsel")
            nc.sync.dma_start(wselB[:], d_wsel.rearrange("(t p) c -> p t c", p=128))
            MSC = sm.tile([128, TC], F32, tag="msc")
            nc.sync.dma_start(MSC[:], d_msc.rearrange("(t p) -> p t", p=128))
            MVC = sm.tile([128, TC], F32, tag="mvc")
            nc.sync.dma_start(MVC[:], d_mvc.rearrange("(t p) -> p t", p=128))
            STC = sm.tile([128, TC], F32, tag="stc")
            nc.sync.dma_start(STC[:], d_stc.rearrange("(t p) -> p t", p=128))
            if flags["bsel"]:
                BSL = sm.tile([128, TC], F32, tag="bsel")
                nc.sync.dma_start(BSL[:], d_bsel.rearrange("(t p) -> p t", p=128))
            if flags["ebq"]:
                EBQ = sm.tile([128, VQ_G * VQ_SIZE], F32, tag="ebq")
                nc.sync.dma_start(EBQ[:], d_ebq.to_broadcast([128, VQ_G * VQ_SIZE]))
            if flags["bspl"]:
                BSP = sm.tile([128, 2], F32, tag="bspl")
                nc.sync.dma_start(BSP[:], d_bspl.to_broadcast([128, 2]))

            SLB = sm.tile([128, TC, 2], F32, tag="SLB")
            GSL = sm.tile([128, TC, VQ_G], F32, tag="GSL")
            TSB = sm.tile([128, TC], F32, tag="TSB")

            # final LN -> XN (bf16); transpose into QT (reused as XNT)
            for t in range(TC):
                with prio(t < 6):
                    layernorm_to(XN[:, t, :], t)
                    for ccc in range(2):
                        nc.sync.dma_start_transpose(
                            QT[:, ccc, t * 128:(t + 1) * 128],
                            XN[:, t, ccc * 128:(ccc + 1) * 128])

            for t in range(TC):
                pse = psc.tile([128, 1024], F32, tag="ps_sc")
                for nk in range(2):
                    for cc in range(2):
                        nc.tensor.matmul(pse[:, nk * 512:(nk + 1) * 512],
                                         QT[:, cc, t * 128:(t + 1) * 128],
                                         wvq[:, cc, nk * 512:(nk + 1) * 512],
                                         start=(cc == 0), stop=(cc == 1))
                EV = evp.tile([128, VQ_G * VQ_SIZE], F32, tag="EV")
                if flags["ebq"]:
                    exp_tile(EV[:], pse[:])
                    nc.vector.tensor_tensor(EV[:], EV[:], EBQ[:],
                                            mybir.AluOpType.mult)
                    g4 = tr.tile([128, VQ_G], F32, tag="g4")
                    nc.vector.tensor_reduce(
                        g4[:],
                        EV[:].rearrange("p (g v) -> p g v", g=VQ_G),
                        mybir.AxisListType.X, mybir.AluOpType.add)
                    nc.gpsimd.tensor_copy(GSL[:, t, :], g4[:])
                elif t % 2 == 0:
                    for gg in range(VQ_G):
                        nc.scalar.activation(
                            EV[:, gg * VQ_SIZE:(gg + 1) * VQ_SIZE],
                            pse[:, gg * VQ_SIZE:(gg + 1) * VQ_SIZE],
                            mybir.ActivationFunctionType.Exp,
                            accum_out=GSL[:, t, gg:gg + 1])
                else:
                    exp_tile(EV[:], pse[:])
                    g4 = tr.tile([128, VQ_G], F32, tag="g4")
                    nc.vector.tensor_reduce(
                        g4[:],
                        EV[:].rearrange("p (g v) -> p g v", g=VQ_G),
                        mybir.AxisListType.X, mybir.AluOpType.add)
                    nc.gpsimd.tensor_copy(GSL[:, t, :], g4[:])

                ps2 = psp.tile([128, 512], F32, tag="bank")
                for cc in range(2):
                    nc.tensor.matmul(ps2[:, :2],
                                     QT[:, cc, t * 128:(t + 1) * 128],
                                     wspl[:, cc, :],
                                     start=(cc == 0), stop=(cc == 1))
                if flags["bspl"]:
                    nc.vector.tensor_tensor(SLB[:, t, :], ps2[:, :2], BSP[:],
                                            mybir.AluOpType.add)
                else:
                    nc.vector.tensor_copy(SLB[:, t, :], ps2[:, :2])

                tmp = tr.tile([128, C], BF16, tag="wdot")
                nc.gpsimd.tensor_tensor(tmp[:], XN[:, t, :], wselB[:, t, :],
                                        mybir.AluOpType.mult)
                rt1 = tr.tile([128, 1], F32, tag="rt1")
                nc.vector.tensor_reduce(rt1[:], tmp[:],
                                        mybir.AxisListType.X, mybir.AluOpType.add)
                nc.gpsimd.tensor_copy(TSB[:, t:t + 1], rt1[:])

            # finish:  ce_v = 0.25*(sum_g ln GSL_g) - 0.25*(TSB [+bsel])
            LGS = sm.tile([128, TC, VQ_G], F32, tag="LGS")
            nc.scalar.activation(
                LGS[:].rearrange("p t g -> p (t g)"),
                GSL[:].rearrange("p t g -> p (t g)"),
                mybir.ActivationFunctionType.Ln)
            CEV = sm.tile([128, TC], F32, tag="CEV")
            nc.vector.tensor_reduce(CEV[:], LGS[:],
                                    mybir.AxisListType.X, mybir.AluOpType.add)
            nc.vector.tensor_sub(CEV[:], CEV[:], TSB[:])
            if flags["bsel"]:
                nc.vector.tensor_sub(CEV[:], CEV[:], BSL[:])
            nc.vector.tensor_scalar_mul(CEV[:], CEV[:], 0.25)

            # ce_s = ln(exp(sl0)+exp(sl1)) - (sl0 + st*(sl1-sl0))
            ES = sm.tile([128, TC, 2], F32, tag="ES")
            nc.scalar.activation(ES[:].rearrange("p t g -> p (t g)"),
                                 SLB[:].rearrange("p t g -> p (t g)"),
                                 mybir.ActivationFunctionType.Exp)
            CES = sm.tile([128, TC], F32, tag="CES")
            nc.vector.tensor_reduce(CES[:], ES[:],
                                    mybir.AxisListType.X, mybir.AluOpType.add)
            nc.scalar.activation(CES[:], CES[:], mybir.ActivationFunctionType.Ln)
            DD = sm.tile([128, TC], F32, tag="DD")
            nc.vector.tensor_sub(DD[:], SLB[:, :, 1], SLB[:, :, 0])
            nc.vector.tensor_tensor(DD[:], DD[:], STC[:], mybir.AluOpType.mult)
            nc.vector.tensor_add(DD[:], DD[:], SLB[:, :, 0])
            nc.vector.tensor_sub(CES[:], CES[:], DD[:])

            R4 = sm.tile([128, 4], F32, tag="R4")
            W1 = sm.tile([128, TC], F32, tag="W1")
            nc.vector.tensor_tensor(W1[:], CES[:], MSC[:], mybir.AluOpType.mult)
            W2 = sm.tile([128, TC], F32, tag="W2")
            nc.vector.tensor_tensor(W2[:], CEV[:], MVC[:], mybir.AluOpType.mult)
            for i, srcbuf in enumerate([W1, MSC, W2, MVC]):
                rtc = tr.tile([128, 1], F32, tag="rtc")
                nc.vector.tensor_reduce(rtc[:], srcbuf[:],
                                        mybir.AxisListType.X, mybir.AluOpType.add)
                nc.vector.tensor_copy(R4[:, i:i + 1], rtc[:])

            nc.sync.dma_start(d_out[:], R4[:])

    nc.compile()
    return nc


def prepare_inputs(inputs):
    """Host-side: fold LN into weights, apply d2b permutation, shard."""
    split = np.asarray(inputs["split"]).astype(np.int64)
    zq = np.asarray(inputs["zq"], dtype=np.float32)
    targets_vq = np.asarray(inputs["targets_vq"]).astype(np.int64)
    category = np.asarray(inputs["category"]).astype(np.int64)
    batch_id = np.asarray(inputs["batch_id"]).astype(np.int64)
    mask = np.asarray(inputs["mask"]).astype(bool)
    d2b = np.asarray(inputs["d2b"]).astype(np.int64)
    g = lambda k: np.asarray(inputs[k], dtype=np.float32)
    split_emb, class_emb = g("split_emb"), g("class_emb")
    vq_proj_w, vq_proj_b = g("vq_proj_w"), g("vq_proj_b")
    ln1_s, ln1_b = g("ln1_s"), g("ln1_b")
    qkv_w, qkv_b = g("qkv_w"), g("qkv_b")
    attn_w, attn_b = g("attn_w"), g("attn_b")
    ln2_s, ln2_b = g("ln2_s"), g("ln2_b")
    fc1_w, fc1_b = g("fc1_w"), g("fc1_b")
    fc2_w, fc2_b = g("fc2_w"), g("fc2_b")
    lnx_s, lnx_b = g("lnx_s"), g("lnx_b")
    split_w, split_b = g("split_w"), g("split_b")
    vq_w, vq_b = g("vq_w"), g("vq_b")

    # LN folds
    qkv_w_eff = ln1_s[:, :, None] * qkv_w                       # [L,C,3C]
    qkv_b_eff = np.einsum("lc,lcn->ln", ln1_b, qkv_w) + qkv_b   # [L,3C]
    fc1_w_eff = ln2_s[:, :, None] * fc1_w
    fc1_b_eff = np.einsum("lc,lcn->ln", ln2_b, fc1_w) + fc1_b
    vq_w_eff = lnx_s[:, None] * vq_w
    vq_b_eff = lnx_b @ vq_w + vq_b
    spl_w_eff = lnx_s[:, None] * split_w
    spl_b_eff = lnx_b @ split_w + split_b

    # fold softmax scale into Q columns
    qkv_w_eff = qkv_w_eff.copy()
    qkv_w_eff[:, :, :C] *= SCALE
    qkv_b_eff = qkv_b_eff.copy()
    qkv_b_eff[:, :C] *= SCALE

    # token embedding pieces, depth order
    cond_rows = class_emb[category[batch_id]]                   # [N,C]
    base_depth = np.empty((N, C), np.float32)
    base_depth[:N_SPLIT] = split_emb[split]
    base_depth[N_SPLIT:] = vq_proj_b[None, :]
    base_depth[mask] = cond_rows[mask]
    zq_depth = np.zeros((N, DH), np.float32)
    zq_depth[N_SPLIT:] = zq
    zq_depth[mask] = 0.0

    ms_depth = np.zeros(N, np.float32)
    ms_depth[:N_SPLIT] = mask[:N_SPLIT]
    mv_depth = np.zeros(N, np.float32)
    mv_depth[N_SPLIT:] = mask[N_SPLIT:]
    st_depth = np.zeros(N, np.float32)
    st_depth[:N_SPLIT] = split
    wsel_depth = np.zeros((N, C), np.float32)
    cols = targets_vq + np.arange(VQ_G)[None, :] * VQ_SIZE      # [N_VQ,4]
    wsel_depth[N_SPLIT:] = vq_w_eff.T[cols].sum(axis=1)         # [N_VQ,C]
    bsel_depth = np.zeros(N, np.float32)
    bsel_depth[N_SPLIT:] = vq_b_eff[cols].sum(axis=1)

    # window order + positional embedding
    pe = _sin_pos_emb(N, C)
    emb_w = base_depth[d2b] + pe
    zq_w = zq_depth[d2b]
    ms_w, mv_w, st_w = ms_depth[d2b], mv_depth[d2b], st_depth[d2b]
    wsel_w, bsel_w = wsel_depth[d2b], bsel_depth[d2b]

    flags = {
        "bqkv": bool(np.any(qkv_b_eff[:, :2 * C])),
        "bqkv_v": bool(np.any(qkv_b_eff[:, 2 * C:])),
        "battn": bool(np.any(attn_b)),
        "bfc1": bool(np.any(fc1_b_eff)),
        "bfc2": bool(np.any(fc2_b)),
        "bspl": bool(np.any(spl_b_eff)),
        "bsel": bool(np.any(bsel_w)),
        "ebq": bool(np.any(vq_b_eff)),
    }

    shared = {
        "vqpw": vq_proj_w.astype(BF),
        "wqkv": qkv_w_eff.reshape(L, 128, 2, 3 * C).astype(F8),
        "wattn": attn_w.reshape(L, 128, 2, C).astype(F8),
        "wfc1": fc1_w_eff.reshape(L, 128, 2, HID).astype(F8),
        "wfc2": fc2_w.astype(F8),
        "bqkv": qkv_b_eff.astype(np.float32),
        "battn": attn_b.astype(np.float32),
        "bfc1": fc1_b_eff.astype(np.float32),
        "bfc2": fc2_b.astype(np.float32),
        "wvq": vq_w_eff.astype(BF),
        "wspl": spl_w_eff.astype(BF),
        "bspl": spl_b_eff.astype(np.float32),
        "ebq": np.exp(vq_b_eff).astype(np.float32),
    }
    in_maps = []
    for c in range(NCORES):
        s = slice(c * T, (c + 1) * T)
        m = dict(shared)
        m["emb"] = np.ascontiguousarray(emb_w[s])
        m["zqt"] = np.ascontiguousarray(zq_w[s].T).astype(BF)
        m["wsel"] = wsel_w[s].astype(BF)
        m["bsel"] = np.ascontiguousarray(bsel_w[s])
        m["msc"] = np.ascontiguousarray(ms_w[s])
        m["mvc"] = np.ascontiguousarray(mv_w[s])
        m["stc"] = np.ascontiguousarray(st_w[s])
        in_maps.append(m)
    return in_maps, flags


def kernel(**inputs) -> np.ndarray:
    in_maps, flags = prepare_inputs(inputs)
    key = tuple(sorted(flags.items()))
    if key not in _CACHE:
        _CACHE[key] = build_nc(flags)
    nc = _CACHE[key]
    res = run_bass_kernel_spmd(nc, in_maps, core_ids=list(range(NCORES)))
    parts = np.stack([res.results[c]["out"].sum(axis=0) for c in range(NCORES)])
    s = parts.sum(axis=0)
    split_loss = s[0] / max(s[1], 1.0)
    vq_loss = s[2] / max(s[3], 1.0)
    return np.stack([split_loss, vq_loss]).astype(np.float32)
